# revision 6
# baseline (speedup 1.0000x reference)
"""Trainium2 Bass kernel for nn_DecoderLSTM: 64-step greedy LSTM decode.

Math (per reference):
    E = emb_table with PAD row zeroed
    h0 = [z, cond] @ W_lh.T + b_lh ;  c0 = 0 ;  x0 = E[SOS]
    per step: gates = x @ W_ih.T + h @ W_hh.T + (b_ih + b_hh)
              i,f,g,o = split(gates); c = sig(f)*c + sig(i)*tanh(g); h = sig(o)*tanh(c)
              logits = h @ W_out.T + b_out ; next x = E[argmax(logits)]

Kernel design (data-parallel over 8 cores, 512 batch rows each):
  * Embedding folded into the gate matmul: x @ W_ih.T == onehot @ G with
    G = E @ W_ih.T (+ bias) precomputed on host -> the decode loop needs no
    gather, just a K=23 matmul against the one-hot of the argmax.
  * Feature-major state: hT/cT stored [H, B] so gatesT[4H-chunk, B] comes
    straight out of the PE (lhsT = W_hh.T chunks, rhs = hT chunks).
  * Logits computed batch-major [B-chunk, 23] by swapping matmul roles
    (lhsT = hT chunk, rhs = W_out.T chunk); b_out added exactly via a K=1
    ones-row matmul accumulated in PSUM.
  * argmax via DVE max/max_index (free dim), one-hot via is_equal against a
    host iota, transposed back to [23, B] with DVE 32x32 stream transposes.
  * All matmuls in fp32 (PE 2-pass fp32: ~6.6e-7 rel accuracy) -- greedy
    argmax trajectories are bit-stable at this precision vs the fp32
    reference (bf16/tf32 flip hundreds of rows and fail).
"""
import numpy as np

import concourse.bass as bass
import concourse.tile as tile
from concourse import bacc, mybir
from concourse.bass_utils import run_bass_kernel_spmd

F32 = mybir.dt.float32
U32 = mybir.dt.uint32

VOCAB, PAD_IDX, SOS_IDX = 23, 21, 22
LATENT, COND, HID, EMB = 128, 16, 1024, 512
MAX_LEN, B, NCORES = 64, 4096, 8
BC = B // NCORES            # 512 batch rows per core
KT = HID // 128             # 8 contraction tiles over H
GT = 4 * HID // 128         # 32 gate M-tiles
NBC = BC // 128             # 4 batch chunks per core
Sig = mybir.ActivationFunctionType.Sigmoid
Tanh = mybir.ActivationFunctionType.Tanh


def build_nc(T=MAX_LEN):
    nc = bacc.Bacc("TRN2", target_bir_lowering=False, debug=False, num_devices=NCORES)

    whhT_d = nc.dram_tensor("whhT", [HID, 4 * HID], F32, kind="ExternalInput")
    g_d = nc.dram_tensor("gmat", [55, 2 * HID], F32, kind="ExternalInput")
    woutT_d = nc.dram_tensor("woutT", [HID, VOCAB], F32, kind="ExternalInput")
    bout_d = nc.dram_tensor("bout", [1, VOCAB], F32, kind="ExternalInput")
    iota_d = nc.dram_tensor("iota", [128, VOCAB], F32, kind="ExternalInput")
    oht0_d = nc.dram_tensor("oht0", [64, BC], F32, kind="ExternalInput")
    h0T_d = nc.dram_tensor("h0T", [HID, BC], F32, kind="ExternalInput")
    out_d = nc.dram_tensor("logits", [T, BC, VOCAB], F32, kind="ExternalOutput")

    with tile.TileContext(nc) as tc:
        with tc.tile_pool(name="state", bufs=1) as sp, \
             tc.tile_pool(name="tmp", bufs=1) as tp, \
             tc.tile_pool(name="lg", bufs=2) as lp, \
             tc.tile_pool(name="gp", bufs=4, space="PSUM") as gp, \
             tc.tile_pool(name="pl", bufs=4, space="PSUM") as pl:

            whh = [sp.tile([128, 4 * HID], F32, tag=f"whh{k}", name=f"whh{k}") for k in range(KT)]
            gmat = sp.tile([55, 2 * HID], F32, tag="gmat")
            wout = sp.tile([128, KT * VOCAB], F32, tag="wout")
            bout = sp.tile([1, VOCAB], F32, tag="bout")
            ones = sp.tile([1, 128], F32, tag="ones")
            iota = sp.tile([128, VOCAB], F32, tag="iota")
            hTs = [sp.tile([128, KT * BC], F32, tag=f"hT{i}", name=f"hT{i}")
                   for i in range(2)]
            cT = sp.tile([128, KT * BC], F32, tag="cT")
            oht = [sp.tile([64, BC], F32, tag=f"oht{i}", name=f"oht{i}") for i in range(2)]
            ohbm = [sp.tile([128, 32], F32, tag=f"ohbm{b}", name=f"ohbm{b}") for b in range(NBC)]

            for k in range(KT):
                nc.sync.dma_start(whh[k][:], whhT_d[128 * k:128 * (k + 1), :])
                nc.sync.dma_start(wout[:, VOCAB * k:VOCAB * (k + 1)],
                                  woutT_d[128 * k:128 * (k + 1), :])
                nc.sync.dma_start(hTs[0][:, BC * k:BC * (k + 1)],
                                  h0T_d[128 * k:128 * (k + 1), :])
            nc.sync.dma_start(gmat[:], g_d[:])
            nc.sync.dma_start(bout[:], bout_d[:])
            nc.sync.dma_start(iota[:], iota_d[:])
            nc.sync.dma_start(oht[0][:], oht0_d[:])
            nc.vector.memset(cT[:], 0.0)
            nc.vector.memset(ones[:], 1.0)
            for b in range(NBC):
                nc.vector.memset(ohbm[b][:], 0.0)

            for t in range(T):
                cur, nxt = oht[t % 2], oht[(t + 1) % 2]
                hT, hTn = hTs[t % 2], hTs[(t + 1) % 2]
                for j in range(KT):
                    ps = []
                    for g4 in range(4):
                        m0 = HID * g4 + 128 * j
                        p = gp.tile([128, BC], F32, tag="gp")
                        for k in range(KT):
                            nc.tensor.matmul(p[:], whh[k][:, m0:m0 + 128],
                                             hT[:, BC * k:BC * (k + 1)],
                                             start=(k == 0), stop=False)
                        if m0 < 2 * HID:
                            nc.tensor.matmul(p[:], gmat[0:VOCAB, m0:m0 + 128],
                                             cur[0:VOCAB, :], start=False, stop=True)
                        else:
                            mo = m0 - 2 * HID
                            nc.tensor.matmul(p[:], gmat[32:32 + VOCAB, mo:mo + 128],
                                             cur[32:32 + VOCAB, :],
                                             start=False, stop=True)
                        ps.append(p)
                    cs = cT[:, BC * j:BC * (j + 1)]
                    hs = hTn[:, BC * j:BC * (j + 1)]
                    tI = tp.tile([128, BC], F32, tag="tI")
                    tG = tp.tile([128, BC], F32, tag="tG")
                    tIG = tp.tile([128, BC], F32, tag="tIG")
                    tF = tp.tile([128, BC], F32, tag="tF")
                    tFC = tp.tile([128, BC], F32, tag="tFC")
                    nc.scalar.activation(tI[:], ps[0][:], Sig)
                    nc.scalar.activation(tG[:], ps[2][:], Tanh)
                    nc.vector.tensor_tensor(tIG[:], tI[:], tG[:], mybir.AluOpType.mult)
                    nc.scalar.activation(tF[:], ps[1][:], Sig)
                    nc.vector.tensor_tensor(tFC[:], tF[:], cs, mybir.AluOpType.mult)
                    nc.vector.tensor_tensor(cs, tIG[:], tFC[:], mybir.AluOpType.add)
                    tTC = tp.tile([128, BC], F32, tag="tI")   # reuse tI slot (dead)
                    tO = tp.tile([128, BC], F32, tag="tG")    # reuse tG slot (dead)
                    nc.scalar.activation(tTC[:], cs, Tanh)
                    nc.scalar.activation(tO[:], ps[3][:], Sig)
                    nc.vector.tensor_tensor(hs, tO[:], tTC[:], mybir.AluOpType.mult)

                for bc in range(NBC):
                    p = pl.tile([128, VOCAB], F32, tag="pl")
                    for k in range(KT):
                        o = BC * k + 128 * bc
                        nc.tensor.matmul(p[:], hTn[:, o:o + 128],
                                         wout[:, VOCAB * k:VOCAB * (k + 1)],
                                         start=(k == 0), stop=False)
                    nc.tensor.matmul(p[:], ones[0:1, :], bout[0:1, :],
                                     start=False, stop=True)
                    lg = lp.tile([128, VOCAB], F32, tag="lg")
                    nc.vector.tensor_copy(lg[:], p[:])
                    nc.sync.dma_start(out_d[t, 128 * bc:128 * (bc + 1), :], lg[:])
                    if t < T - 1:
                        mx = lp.tile([128, 8], F32, tag="mx")
                        nc.vector.max(mx[:], lg[:, 0:VOCAB])
                        ix = lp.tile([128, 8], U32, tag="ix")
                        nc.vector.max_index(ix[:], mx[:], lg[:, 0:VOCAB])
                        ixf = lp.tile([128, 1], F32, tag="ixf")
                        nc.vector.tensor_copy(ixf[:], ix[:, 0:1])
                        nc.vector.tensor_scalar(ohbm[bc][:, 0:VOCAB], iota[:], ixf[:],
                                                None, mybir.AluOpType.is_equal)
                        for a in range(4):
                            src_ap = ohbm[bc][32 * a:32 * (a + 1), 0:32]
                            nc.vector.transpose(
                                nxt[0:32, 128 * bc + 32 * a:128 * bc + 32 * (a + 1)],
                                src_ap)
                            nc.vector.transpose(
                                nxt[32:64, 128 * bc + 32 * a:128 * bc + 32 * (a + 1)],
                                src_ap)

    nc.compile()
    return nc


def host_prep(inputs, T=MAX_LEN):
    f = lambda k: np.asarray(inputs[k], dtype=np.float32)
    E = f("emb_table").copy()
    E[PAD_IDX] = 0.0
    bias = f("b_ih") + f("b_hh")
    G = E @ f("W_ih").T + bias                       # [V, 4H], bias folded
    Gp = np.zeros((55, 2 * HID), dtype=np.float32)   # two partition stacks
    Gp[0:VOCAB] = G[:, 0:2 * HID]
    Gp[32:32 + VOCAB] = G[:, 2 * HID:4 * HID]
    h0 = np.concatenate([f("z"), f("cond")], axis=1) @ f("W_lh").T + f("b_lh")
    common = {
        "whhT": np.ascontiguousarray(f("W_hh").T),
        "gmat": Gp,
        "woutT": np.ascontiguousarray(f("W_out").T),
        "bout": np.ascontiguousarray(f("b_out")[None, :]),
        "iota": np.broadcast_to(np.arange(VOCAB, dtype=np.float32),
                                (128, VOCAB)).copy(),
        "oht0": np.zeros((64, BC), dtype=np.float32),
    }
    common["oht0"][SOS_IDX] = 1.0
    common["oht0"][32 + SOS_IDX] = 1.0
    in_maps = []
    for c in range(NCORES):
        m = dict(common)
        m["h0T"] = np.ascontiguousarray(h0[BC * c:BC * (c + 1)].T)
        in_maps.append(m)
    return in_maps


_NC_CACHE = {}


def _get_nc(T=MAX_LEN):
    if T not in _NC_CACHE:
        _NC_CACHE[T] = build_nc(T)
    return _NC_CACHE[T]


def kernel(**inputs) -> np.ndarray:
    nc = _get_nc(MAX_LEN)
    in_maps = host_prep(inputs, MAX_LEN)
    res = run_bass_kernel_spmd(nc, in_maps, core_ids=list(range(NCORES)))
    out = np.empty((B, MAX_LEN, VOCAB), dtype=np.float32)
    for c in range(NCORES):
        # per-core result: [T, BC, V] -> [BC, T, V]
        out[BC * c:BC * (c + 1)] = res.results[c]["logits"].transpose(1, 0, 2)
    return out
